# revision 21
# baseline (speedup 1.0000x reference)
"""Trainium2 Bass kernel for nn_ChemResBlock (gnn_message_passing).

Reference computation (A=2048 atoms, D=64 depth, F=12 filter slots):
    chemconv(x)[a,o] = sum_{n,f,d} conn[a,n,f] * x[n,d] * filters[o,f,d]
                       + sum_{f,c} bond[a,f,c] * filters[o,f,D+c]
    for filt in (f0, f1):
        out = relu(chemconv(out)); out = chemconv(out); out = relu(out + x)

Kernel strategy (8 NeuronCores), v3:
  * Contract-reorder: z[o,a] = sum_{n,f} conn_t[(f,n),a] * y[(f,n),o] with
    y = x @ filters — the big conn tensor is consumed by matmuls.
  * Shard the contraction (neighbor) dim across 8 cores; each core owns a
    PERMUTED set of 256 atoms: slices [128c, 128c+128) and [1024+128c, +128),
    so the two per-conv half-ReduceScatters each deliver one 128-atom slice.
  * fp16 everywhere on the fast path (conn, y, z-drain, RS payload): same
    speed/bytes as bf16 but 4x finer mantissa — keeps the error budget far
    under the gate.
  * Col-tiled matmuls: chunk PAIRS run CONCURRENTLY on disjoint PE column
    halves (tile_position (0,0) / (0,64)), each accumulating into its own
    psum partition half; drain adds the halves. ~2x PE throughput since
    M=64 only uses half the 128-wide array.
  * A dummy 4-element AllReduce issues at t~0 so the one-time CC-init
    barrier (~80us!) overlaps the conn load + conv0 instead of serializing
    before conv0's first ReduceScatter.
  * Per conv, g-major passes: all chunks for column half g, then drain ->
    one consolidated DMA store -> ReduceScatter(g), so RS(g0) overlaps the
    g1 MM pass. Next conv's even (ns0) chunks run as soon as RS(g0) lands.
  * Engine queues: sync = conn loads + post loads + out stores; scalar =
    small loads + cc_in stores; vector = drains/bias/residual/relu;
    gpsimd = collectives.  Output = per-core [64, 256] f32 shards
    (permuted atom ownership), reassembled host-side.
"""

import numpy as np
import ml_dtypes

import concourse.bacc as bacc
import concourse.bass as bass
import concourse.mybir as mybir
import concourse.tile as tile
from concourse.bass_utils import run_bass_kernel_spmd

A, D, F, NCORES = 2048, 64, 12, 8
NS = A // NCORES          # owned atoms per core = 256
KL = NS * F               # local contraction size = 3072
NCH = KL // 128           # k-chunks of 128 = 24
FO = F * D                # 768 = y columns per layer
CH = A // 2               # column half = 1024

FP = mybir.dt.float32
HF = mybir.dt.float16
E3 = mybir.dt.float8e3
NPHF = np.float16
NPE3 = ml_dtypes.float8_e3m4

# Per-conv downscale so fp16 intermediates (z partials on the RS wire) stay
# in range: activations grow ~350x per conv; conv3 pre-activations reach
# ~5.7e8 >> fp16 max. The total wire scale SCALES[conv] is split between the
# x-storage side (XSIN: x_next stored as x/XSIN) and the fw side (FWDIV kept
# <=4 so fp16 fw weights stay out of denormal/FTZ range). Un-scaled in the
# post step right after the ReduceScatter.
# conn is stored as fp8e3 * 8 (top of e3m4 normal range; immune to any HW
# subnormal flush below 0.25/8=0.03 true units), so every wire value carries
# an extra 1/8; SCALES includes that factor.
CONN_SCALE = 8.0
SCALES = [1.0, 1.0, 128.0, 65536.0]
XSIN = [1.0, 1.0, 64.0, 32768.0]
# z_wire = CONN_SCALE * z_true / (XSIN * FWDIV)  ==  z_true / SCALES
FWDIV = [s * CONN_SCALE / x for s, x in zip(SCALES, XSIN)]

_CACHE = {}


def _build():
    nc = bacc.Bacc("TRN2", target_bir_lowering=False, debug=False, num_devices=NCORES)

    conn_t_d = nc.dram_tensor("conn_t", [KL, A], E3, kind="ExternalInput").ap()
    xoT_d = nc.dram_tensor("xoT_sh", [D, NS], FP, kind="ExternalInput").ap()
    fw_d = nc.dram_tensor("fw", [D, 4 * FO], HF, kind="ExternalInput").ap()
    fb_d = nc.dram_tensor("fb", [2 * F, 2 * D], HF, kind="ExternalInput").ap()
    bondT_d = nc.dram_tensor("bondT_sh", [2 * F, NS], HF, kind="ExternalInput").ap()
    out_d = nc.dram_tensor("out_sh", [D, NS], FP, kind="ExternalOutput").ap()

    with tile.TileContext(nc) as tc:
        with (
            tc.tile_pool(name="res", bufs=1) as res_pool,
            tc.tile_pool(name="sb", bufs=1) as sb,
            tc.tile_pool(name="ypool", bufs=1) as ypool,
            tc.tile_pool(name="ztpool", bufs=2) as ztpool,
            tc.tile_pool(name="work", bufs=2) as work,
            tc.tile_pool(name="psy", bufs=2, space="PSUM") as psy,
            tc.tile_pool(name="psz", bufs=2, space="PSUM") as psz,
            tc.tile_pool(name="dram", bufs=1, space="DRAM") as dram,
        ):
            # ---- small tensors on the scalar (ACT) ring ----
            xoT_sb = sb.tile([D, NS], FP, name="xoT_sb", tag="xoT_sb")
            nc.scalar.dma_start(xoT_sb[:], xoT_d)
            fw_sb = sb.tile([D, 4 * FO], HF, name="fw_sb", tag="fw_sb")
            nc.scalar.dma_start(fw_sb[:], fw_d)
            fb_sb = sb.tile([2 * F, 2 * D], HF, name="fb_sb", tag="fb_sb")
            nc.scalar.dma_start(fb_sb[:], fb_d)
            bondT_sb = sb.tile([2 * F, NS], HF, name="bondT_sb", tag="bondT_sb")
            nc.scalar.dma_start(bondT_sb[:], bondT_d)

            # ---- conn shard: 48 half tiles [128, 1024] fp8e3, g0 halves
            # first across sync+gpsimd queues, so conv0's g0 pass (and its
            # ReduceScatter trigger) completes after only 3.15MB of load.
            # The first-RS start time is last_core_start + conv0 critical
            # path (the CC rendezvous absorbs launch skew), so conv0 speed
            # directly moves the whole schedule. ----
            conn_res = [[None, None] for _ in range(NCH)]
            for g in range(2):
                for r in range(NCH):
                    conn_res[r][g] = res_pool.tile(
                        [128, CH], E3, name=f"cn{r}_{g}", tag=f"cn{r}_{g}")
            # all conn on the sync queue: its DMA-ring pool (163-172) is
            # shared with scalar, while gpsimd owns rings 155-162 — keeping
            # gpsimd conn-free lets the cc_in stores (and so the RS
            # triggers) complete without queueing behind conn traffic.
            for g in range(2):
                for r in range(NCH):
                    eng = nc.scalar if (g == 0 and r % 3 == 2) else nc.sync
                    eng.dma_start(
                        conn_res[r][g][:],
                        conn_t_d[r * 128:(r + 1) * 128, g * CH:(g + 1) * CH])

            # per-conv combined bias cb[conv] = (bias_layer + x?residual)/XSIN_next
            # so each post step is a single scalar_tensor_tensor + relu.
            # Emitted lazily (at conv0's first post) so the pb matmuls don't
            # sit ahead of conv0's z matmuls in the PE queue — they only
            # need to land before the first RS completes (~60us later).
            cb_sb = sb.tile([D, 4, NS], FP, name="cb_sb", tag="cb_sb")
            cb_state = {"done": False}

            def emit_cb():
                if cb_state["done"]:
                    return
                cb_state["done"] = True
                for layer in range(2):
                    pb = psy.tile([D, NS], FP, name="pb", tag="py")
                    nc.tensor.matmul(
                        pb[:], fb_sb[:, layer * D:(layer + 1) * D], bondT_sb[:],
                        start=True, stop=True,
                    )
                    ceven, codd = 2 * layer, 2 * layer + 1
                    nc.vector.tensor_scalar(
                        cb_sb[:, ceven, :], pb[:], 1.0 / XSIN[ceven + 1], None,
                        op0=mybir.AluOpType.mult)
                    xs_n = XSIN[codd + 1] if codd < 3 else 1.0
                    tmpb = work.tile([D, NS], FP, name="tmpb", tag="tmpb")
                    nc.vector.tensor_add(tmpb[:], pb[:], xoT_sb[:])
                    nc.vector.tensor_scalar(
                        cb_sb[:, codd, :], tmpb[:], 1.0 / xs_n, None,
                        op0=mybir.AluOpType.mult)

            # fp16 copy of x-transposed (conv0 y production)
            xo_hf = sb.tile([D, NS], HF, name="xo_hf", tag="xo_hf")
            nc.vector.tensor_copy(xo_hf[:], xoT_sb[:])

            y_tiles = [
                ypool.tile([128, 2, FO], HF, name=f"y{i}", tag=f"y{i}")
                for i in range(2)
            ]
            nxt_hf = [
                sb.tile([D, NS], HF, name=f"nb{i}", tag=f"nb{i}")
                for i in range(2)
            ]

            def y_emit(conv, blk, cur_hf):
                """y[n_local(blk), f, o] for this conv (pre-scaled fw copy)."""
                y_t = y_tiles[conv % 2]
                for h in range(2):
                    py = psy.tile([128, FO // 2], FP, name="py", tag="py")
                    nc.tensor.matmul(
                        py[:],
                        cur_hf[:, blk * 128:(blk + 1) * 128],
                        fw_sb[:, conv * FO + h * (FO // 2):
                              conv * FO + (h + 1) * (FO // 2)],
                        start=True, stop=True,
                    )
                    if h == 0:
                        nc.vector.tensor_copy(
                            y_t[:, blk, h * (FO // 2):(h + 1) * (FO // 2)],
                            py[:])
                    else:
                        nc.scalar.copy(
                            y_t[:, blk, h * (FO // 2):(h + 1) * (FO // 2)],
                            py[:])

            # conv0's block-0 y from x itself (block 1 woven mid-pass so
            # z matmuls start as early as possible after engine init)
            y_emit(0, 0, xo_hf)

            cc_in = [[None, None] for _ in range(4)]
            cc_out = [[None, None] for _ in range(4)]
            for c in range(4):
                for g in range(2):
                    cc_in[c][g] = dram.tile(
                        [NCORES, D, 128], HF, name=f"ci{c}{g}", tag=f"ci{c}{g}")
                    cc_out[c][g] = dram.tile(
                        [D, 128], HF, name=f"co{c}{g}", tag=f"co{c}{g}")

            scope = nc.named_scope
            for conv in range(4):
                layer = conv // 2
                y_t = y_tiles[conv % 2]
                if conv > 0:
                    # block-0 y (depends on prior conv's first RS result);
                    # lands in the PE queue after the prior conv's pass-1 MMs
                    y_emit(conv, 0, nxt_hf[(conv - 1) % 2])

                sc = scope(f"conv{conv}"); sc.__enter__()
                for g in range(2):
                    pz = [
                        psz.tile([128, 512], FP, name=f"pz{i}", tag=f"pz{i}")
                        for i in range(2)
                    ]
                    # chunk-pair order: even (ns0) chunks first so the conv
                    # can start on y-block-0 only; None marker weaves the
                    # block-1 y production mid-pass (from x for conv0, from
                    # the second RS half otherwise).
                    ev = [r for r in range(NCH) if r % 2 == 0]
                    od = [r for r in range(NCH) if r % 2 == 1]
                    order = [(ev[2 * j], ev[2 * j + 1])
                             for j in range(len(ev) // 2)]
                    if g == 0:
                        order = order + [None]
                    order = order + [(od[2 * j], od[2 * j + 1])
                                     for j in range(len(od) // 2)]
                    npair = NCH // 2
                    half = npair // 2

                    def z_mm(pr, i, first, last):
                        cs = slice(i * 512, (i + 1) * 512)
                        rA, rB = pr
                        fA, nsA = rA // 2, rA % 2
                        fB, nsB = rB // 2, rB % 2
                        for tp, rX, fX, nsX in ((0, rA, fA, nsA),
                                                (64, rB, fB, nsB)):
                            nc.tensor.matmul(
                                pz[i][tp:tp + 64, :],
                                y_t[:, nsX, fX * D:(fX + 1) * D],
                                conn_res[rX][g][:, cs],
                                start=first, stop=last,
                                tile_position=(0, tp),
                                skip_group_check=True,
                            )

                    # first-half pairs: both column quarters interleaved
                    # (these run while the prior RS halves are in flight);
                    # then y1 weave; then second-half pairs quarter-major so
                    # quarter 0's drain+store overlaps quarter 1's matmuls.
                    pairs = [pr for pr in order if pr is not None]
                    for pi, pr in enumerate(pairs[:half]):
                        for i in range(2):
                            z_mm(pr, i, first=(pi == 0), last=False)
                    if None in order:
                        y_emit(conv, 1,
                               xo_hf if conv == 0 else nxt_hf[(conv - 1) % 2])
                    for i in range(2):
                        for pi, pr in enumerate(pairs[half:]):
                            z_mm(pr, i, first=False,
                                 last=(pi == half - 1 if half else False))

                    # ---- drain per column-quarter i: i=0's drain+store
                    # overlaps i=1's matmuls. HW allows only ONE PSUM input
                    # per tensor-tensor op: scalar-copy the top col-tile
                    # half to SBUF, then vector-add PSUM + SBUF, then store
                    # ranks j=4i..4i+3 (on gpsimd, same queue as the RS so
                    # the trigger fences on it directly). ----
                    zt = ztpool.tile([D, CH], HF, name=f"zt{g}", tag=f"zt{g}")
                    ph = work.tile([D, 2, 512], HF, name=f"ph{g}", tag=f"ph{g}")
                    for i in range(2):
                        nc.scalar.copy(ph[:, i, :], pz[i][64:128, :])
                        nc.vector.tensor_add(
                            zt[:, i * 512:(i + 1) * 512],
                            pz[i][0:64, :], ph[:, i, :])
                        for jj in range(2):
                            j0 = 4 * i + 2 * jj
                            nc.gpsimd.dma_start(
                                cc_in[conv][g][j0:j0 + 2].transpose([1, 0, 2]),
                                zt[:, j0 * 128:(j0 + 2) * 128])

                    scc = scope(f"cc{conv}g{g}"); scc.__enter__()
                    nc.gpsimd.collective_compute(
                        "ReduceScatter",
                        mybir.AluOpType.add,
                        replica_groups=[list(range(NCORES))],
                        ins=[cc_in[conv][g].opt()],
                        outs=[cc_out[conv][g].opt()],
                    )
                    scc.__exit__(None, None, None)

                    # ---- post for owned block g: bias + (residual) + relu ----
                    gs = slice(g * 128, (g + 1) * 128)
                    emit_cb()
                    sl = work.tile([D, 128], HF, name=f"sl{g}", tag=f"sl{g}")
                    nc.sync.dma_start(sl[:, 0:64], cc_out[conv][g][:, 0:64])
                    nc.scalar.dma_start(sl[:, 64:128], cc_out[conv][g][:, 64:128])
                    t1 = work.tile([D, 128], FP, name=f"t1{g}", tag=f"t1{g}")
                    # t1 = sl * (SCALES/XSIN_next) + cb  (cb folds bias,
                    # residual x, and the x_next storage scale)
                    xs_n = XSIN[conv + 1] if conv < 3 else 1.0
                    nc.vector.scalar_tensor_tensor(
                        t1[:], sl[:], float(SCALES[conv] / xs_n),
                        cb_sb[:, conv, gs],
                        op0=mybir.AluOpType.mult, op1=mybir.AluOpType.add)
                    if conv == 3:
                        nf = work.tile([D, 128], FP, name=f"nf{g}", tag=f"nf{g}")
                        nc.vector.tensor_scalar_max(nf[:], t1[:], 0.0)
                        nc.sync.dma_start(out_d[:, gs], nf[:])
                    else:
                        nc.vector.tensor_scalar_max(
                            nxt_hf[conv % 2][:, gs], t1[:], 0.0)
                sc.__exit__(None, None, None)

    nc.compile()
    return nc


def _get_nc():
    if "nc" not in _CACHE:
        _CACHE["nc"] = _build()
    return _CACHE["nc"]


def _owned(c):
    return np.r_[128 * c:128 * c + 128, 1024 + 128 * c:1024 + 128 * c + 128]


def _prep_in_maps(inputs):
    x = np.ascontiguousarray(inputs["node_property_tensor"], dtype=np.float32)
    conn = np.ascontiguousarray(inputs["connectivity_tensor"], dtype=np.float32)
    bond = np.ascontiguousarray(inputs["bond_property_tensor"], dtype=np.float32)
    f0 = np.ascontiguousarray(inputs["filters0"], dtype=np.float32)
    f1 = np.ascontiguousarray(inputs["filters1"], dtype=np.float32)

    xT = np.ascontiguousarray(x.T)                                   # [D, A]
    fws = [f[:, :, :D].transpose(2, 1, 0).reshape(D, FO) for f in (f0, f1)]
    fw = np.concatenate(
        [fws[conv // 2] / FWDIV[conv] for conv in range(4)],
        axis=1).astype(NPHF)                                         # [D, 4*FO]
    fb = np.ascontiguousarray(np.concatenate(
        [f[:, :, D:].reshape(D, 2 * F).T for f in (f0, f1)],
        axis=1).astype(NPHF))
    bondT = np.ascontiguousarray(
        bond.transpose(1, 2, 0).reshape(2 * F, A).astype(NPHF))

    in_maps = []
    for c in range(NCORES):
        own = _owned(c)
        conn_t = np.ascontiguousarray(
            (conn[:, own, :].transpose(2, 1, 0).reshape(KL, A)
             * CONN_SCALE).astype(NPE3))
        in_maps.append({
            "conn_t": conn_t,
            "xoT_sh": np.ascontiguousarray(xT[:, own]),
            "fw": np.ascontiguousarray(fw),
            "fb": fb,
            "bondT_sh": np.ascontiguousarray(bondT[:, own]),
        })
    return in_maps


def _unshard(results):
    outT = np.empty((D, A), dtype=np.float32)
    for c in range(NCORES):
        sh = results[c]["out_sh"]
        outT[:, 128 * c:128 * c + 128] = sh[:, :128]
        outT[:, 1024 + 128 * c:1024 + 128 * c + 128] = sh[:, 128:]
    return np.ascontiguousarray(outT.T)


def kernel(node_property_tensor, connectivity_tensor, bond_property_tensor,
           filters0, filters1):
    in_maps = _prep_in_maps({
        "node_property_tensor": node_property_tensor,
        "connectivity_tensor": connectivity_tensor,
        "bond_property_tensor": bond_property_tensor,
        "filters0": filters0,
        "filters1": filters1,
    })
    nc = _get_nc()
    res = run_bass_kernel_spmd(nc, in_maps, core_ids=list(range(NCORES)))
    return _unshard(res.results)


def run_traced(in_maps, stitch=False):
    """For test.py: run with NTFF tracing, return BassKernelResults."""
    kw = {}
    if stitch:
        kw = dict(trace_cores=list(range(NCORES)), stitch_traces=True)
    return run_bass_kernel_spmd(
        _get_nc(), in_maps, core_ids=list(range(NCORES)), trace=True, **kw
    )


def make_in_maps(**inputs):
    """Expose the host-side prep for test.py tracing path."""
    return _prep_in_maps(inputs)


# revision 24
# speedup vs baseline: 1.0743x; 1.0743x over previous
"""Trainium2 Bass kernel for nn_ChemResBlock (gnn_message_passing).

Reference computation (A=2048 atoms, D=64 depth, F=12 filter slots):
    chemconv(x)[a,o] = sum_{n,f,d} conn[a,n,f] * x[n,d] * filters[o,f,d]
                       + sum_{f,c} bond[a,f,c] * filters[o,f,D+c]
    for filt in (f0, f1):
        out = relu(chemconv(out)); out = chemconv(out); out = relu(out + x)

Kernel strategy (8 NeuronCores), v7:
  * Contract-reorder: z[o,a] = sum_{n,f} conn_t[(f,n),a] * y[(f,n),o] with
    y = x @ filters — the big conn tensor is consumed by matmuls.
  * Shard the contraction (neighbor) dim across 8 cores; each core owns a
    PERMUTED set of 256 atoms: slices [128c, 128c+128) and [1024+128c, +128),
    so the two per-conv half-ReduceScatters each deliver one 128-atom slice.
  * conn stored fp8e3 * 8 (top of e3m4 normal range, immune to subnormal
    flush): halves HBM traffic + SBUF vs fp16 at the same matmul rate.
    y / z-drain / RS wire in fp16 (4x finer mantissa than bf16). Per-conv
    scales (SCALES/XSIN/FWDIV) keep fp16 wire values in range while fw
    weights stay out of denormal territory; HW rel err 2.1e-3 vs 2e-2 gate.
  * Col-tiled matmuls: chunk PAIRS run CONCURRENTLY on disjoint PE column
    halves (tile_position (0,0)/(0,64)), ~1.9x since M=64 only fills half
    the 128-wide array. Per column-quarter psum chains so quarter 0's
    drain+store overlaps quarter 1's matmuls.
  * Per conv, g-major passes: chunks for column half g (even ns0 chunks
    first, block-1 y woven mid-pass), drain -> rank-sliced stores ->
    ReduceScatter(g). RS(g0) overlaps the g1 pass; the next conv's even
    chunks run under RS(g1).
  * First-RS time = last_core_start + conv0 critical path (the CC-init
    rendezvous absorbs NEFF launch skew, then a fixed ~11us first-op
    penalty): conv0 is tuned to trigger ASAP (~37us). The combined-bias
    matmuls are deferred past conv0's z matmuls in the PE queue.
  * DMA ring discipline: sync+scalar share rings 163-172, gpsimd owns
    155-162. conn loads live on sync only; cc_in stores (4 x 32KB per
    half-conv) + RS triggers on gpsimd so they never queue behind conn.
  * Post per owned block: one scalar_tensor_tensor (wire*scale + combined
    bias incl residual) + relu(+x_next storage scale); final conv stores
    f32 shards [64, 256] (permuted atom ownership), reassembled host-side.
"""

import numpy as np
import ml_dtypes

import concourse.bacc as bacc
import concourse.bass as bass
import concourse.mybir as mybir
import concourse.tile as tile
from concourse.bass_utils import run_bass_kernel_spmd

A, D, F, NCORES = 2048, 64, 12, 8
NS = A // NCORES          # owned atoms per core = 256
KL = NS * F               # local contraction size = 3072
NCH = KL // 128           # k-chunks of 128 = 24
FO = F * D                # 768 = y columns per layer
CH = A // 2               # column half = 1024

FP = mybir.dt.float32
HF = mybir.dt.float16
E3 = mybir.dt.float8e3
NPHF = np.float16
NPE3 = ml_dtypes.float8_e3m4

# Per-conv downscale so fp16 intermediates (z partials on the RS wire) stay
# in range: activations grow ~350x per conv; conv3 pre-activations reach
# ~5.7e8 >> fp16 max. The total wire scale SCALES[conv] is split between the
# x-storage side (XSIN: x_next stored as x/XSIN) and the fw side (FWDIV kept
# <=4 so fp16 fw weights stay out of denormal/FTZ range). Un-scaled in the
# post step right after the ReduceScatter.
# conn is stored as fp8e3 * 8 (top of e3m4 normal range; immune to any HW
# subnormal flush below 0.25/8=0.03 true units), so every wire value carries
# an extra 1/8; SCALES includes that factor.
CONN_SCALE = 8.0
SCALES = [1.0, 1.0, 128.0, 65536.0]
XSIN = [1.0, 1.0, 64.0, 32768.0]
# z_wire = CONN_SCALE * z_true / (XSIN * FWDIV)  ==  z_true / SCALES
FWDIV = [s * CONN_SCALE / x for s, x in zip(SCALES, XSIN)]

_CACHE = {}


def _build():
    nc = bacc.Bacc("TRN2", target_bir_lowering=False, debug=False, num_devices=NCORES)

    conn_t_d = nc.dram_tensor("conn_t", [KL, A], E3, kind="ExternalInput").ap()
    xoT_d = nc.dram_tensor("xoT_sh", [D, NS], FP, kind="ExternalInput").ap()
    fw_d = nc.dram_tensor("fw", [D, 4 * FO], HF, kind="ExternalInput").ap()
    fb_d = nc.dram_tensor("fb", [2 * F, 2 * D], HF, kind="ExternalInput").ap()
    bondT_d = nc.dram_tensor("bondT_sh", [2 * F, NS], HF, kind="ExternalInput").ap()
    out_d = nc.dram_tensor("out_sh", [D, NS], FP, kind="ExternalOutput").ap()

    with tile.TileContext(nc) as tc:
        with (
            tc.tile_pool(name="res", bufs=1) as res_pool,
            tc.tile_pool(name="sb", bufs=1) as sb,
            tc.tile_pool(name="ypool", bufs=1) as ypool,
            tc.tile_pool(name="ztpool", bufs=2) as ztpool,
            tc.tile_pool(name="work", bufs=2) as work,
            tc.tile_pool(name="psy", bufs=2, space="PSUM") as psy,
            tc.tile_pool(name="psz", bufs=2, space="PSUM") as psz,
            tc.tile_pool(name="dram", bufs=1, space="DRAM") as dram,
        ):
            # ---- small tensors on the scalar (ACT) ring ----
            xoT_sb = sb.tile([D, NS], FP, name="xoT_sb", tag="xoT_sb")
            nc.scalar.dma_start(xoT_sb[:], xoT_d)
            fw_sb = sb.tile([D, 4 * FO], HF, name="fw_sb", tag="fw_sb")
            nc.scalar.dma_start(fw_sb[:], fw_d)
            fb_sb = sb.tile([2 * F, 2 * D], HF, name="fb_sb", tag="fb_sb")
            nc.scalar.dma_start(fb_sb[:], fb_d)
            bondT_sb = sb.tile([2 * F, NS], HF, name="bondT_sb", tag="bondT_sb")
            nc.scalar.dma_start(bondT_sb[:], bondT_d)

            # ---- conn shard: 48 half tiles [128, 1024] fp8e3, g0 halves
            # first, so conv0's g0 pass (and its ReduceScatter trigger)
            # completes after only 3.15MB of load. The first-RS start time
            # is last_core_start + conv0 critical path (the CC rendezvous
            # absorbs launch skew), so conv0 speed directly moves the
            # whole schedule. ----
            conn_res = [[None, None] for _ in range(NCH)]
            for g in range(2):
                for r in range(NCH):
                    conn_res[r][g] = res_pool.tile(
                        [128, CH], E3, name=f"cn{r}_{g}", tag=f"cn{r}_{g}")
            # all conn on the sync queue: its DMA-ring pool (163-172) is
            # shared with scalar, while gpsimd owns rings 155-162 — keeping
            # gpsimd conn-free lets the cc_in stores (and so the RS
            # triggers) complete without queueing behind conn traffic.
            for g in range(2):
                for r in range(NCH):
                    nc.sync.dma_start(
                        conn_res[r][g][:],
                        conn_t_d[r * 128:(r + 1) * 128, g * CH:(g + 1) * CH])

            # per-conv combined bias cb[conv] = (bias_layer + x?residual)/XSIN_next
            # so each post step is a single scalar_tensor_tensor + relu.
            # Emitted lazily (at conv0's first post) so the pb matmuls don't
            # sit ahead of conv0's z matmuls in the PE queue — they only
            # need to land before the first RS completes (~60us later).
            cb_sb = sb.tile([D, 4, NS], FP, name="cb_sb", tag="cb_sb")
            cb_state = {"done": False}

            def emit_cb():
                if cb_state["done"]:
                    return
                cb_state["done"] = True
                for layer in range(2):
                    pb = psy.tile([D, NS], FP, name="pb", tag="py")
                    nc.tensor.matmul(
                        pb[:], fb_sb[:, layer * D:(layer + 1) * D], bondT_sb[:],
                        start=True, stop=True,
                    )
                    ceven, codd = 2 * layer, 2 * layer + 1
                    nc.vector.tensor_scalar(
                        cb_sb[:, ceven, :], pb[:], 1.0 / XSIN[ceven + 1], None,
                        op0=mybir.AluOpType.mult)
                    xs_n = XSIN[codd + 1] if codd < 3 else 1.0
                    tmpb = work.tile([D, NS], FP, name="tmpb", tag="tmpb")
                    nc.vector.tensor_add(tmpb[:], pb[:], xoT_sb[:])
                    nc.vector.tensor_scalar(
                        cb_sb[:, codd, :], tmpb[:], 1.0 / xs_n, None,
                        op0=mybir.AluOpType.mult)

            # fp16 copy of x-transposed (conv0 y production)
            xo_hf = sb.tile([D, NS], HF, name="xo_hf", tag="xo_hf")
            nc.vector.tensor_copy(xo_hf[:], xoT_sb[:])

            y_tiles = [
                ypool.tile([128, 2, FO], HF, name=f"y{i}", tag=f"y{i}")
                for i in range(2)
            ]
            nxt_hf = [
                sb.tile([D, NS], HF, name=f"nb{i}", tag=f"nb{i}")
                for i in range(2)
            ]

            def y_emit(conv, blk, cur_hf):
                """y[n_local(blk), f, o] for this conv (pre-scaled fw copy)."""
                y_t = y_tiles[conv % 2]
                for h in range(2):
                    py = psy.tile([128, FO // 2], FP, name="py", tag="py")
                    nc.tensor.matmul(
                        py[:],
                        cur_hf[:, blk * 128:(blk + 1) * 128],
                        fw_sb[:, conv * FO + h * (FO // 2):
                              conv * FO + (h + 1) * (FO // 2)],
                        start=True, stop=True,
                    )
                    nc.scalar.copy(
                        y_t[:, blk, h * (FO // 2):(h + 1) * (FO // 2)], py[:])

            # conv0's block-0 y from x itself (block 1 woven mid-pass so
            # z matmuls start as early as possible after engine init)
            y_emit(0, 0, xo_hf)

            cc_in = [[None, None] for _ in range(4)]
            cc_out = [[None, None] for _ in range(4)]
            for c in range(4):
                for g in range(2):
                    cc_in[c][g] = dram.tile(
                        [NCORES, D, 128], HF, name=f"ci{c}{g}", tag=f"ci{c}{g}")
                    cc_out[c][g] = dram.tile(
                        [D, 128], HF, name=f"co{c}{g}", tag=f"co{c}{g}")

            scope = nc.named_scope
            for conv in range(4):
                layer = conv // 2
                y_t = y_tiles[conv % 2]
                if conv > 0:
                    # block-0 y (depends on prior conv's first RS result);
                    # lands in the PE queue after the prior conv's pass-1 MMs
                    y_emit(conv, 0, nxt_hf[(conv - 1) % 2])

                sc = scope(f"conv{conv}"); sc.__enter__()
                for g in range(2):
                    pz = [
                        psz.tile([128, 512], FP, name=f"pz{i}", tag=f"pz{i}")
                        for i in range(2)
                    ]
                    # chunk-pair order: even (ns0) chunks first so the conv
                    # can start on y-block-0 only; None marker weaves the
                    # block-1 y production mid-pass (from x for conv0, from
                    # the second RS half otherwise).
                    ev = [r for r in range(NCH) if r % 2 == 0]
                    od = [r for r in range(NCH) if r % 2 == 1]
                    order = [(ev[2 * j], ev[2 * j + 1])
                             for j in range(len(ev) // 2)]
                    if g == 0:
                        order = order + [None]
                    order = order + [(od[2 * j], od[2 * j + 1])
                                     for j in range(len(od) // 2)]
                    npair = NCH // 2
                    half = npair // 2

                    def z_mm(pr, i, first, last):
                        cs = slice(i * 512, (i + 1) * 512)
                        rA, rB = pr
                        fA, nsA = rA // 2, rA % 2
                        fB, nsB = rB // 2, rB % 2
                        for tp, rX, fX, nsX in ((0, rA, fA, nsA),
                                                (64, rB, fB, nsB)):
                            nc.tensor.matmul(
                                pz[i][tp:tp + 64, :],
                                y_t[:, nsX, fX * D:(fX + 1) * D],
                                conn_res[rX][g][:, cs],
                                start=first, stop=last,
                                tile_position=(0, tp),
                                skip_group_check=True,
                            )

                    # first-half pairs: both column quarters interleaved
                    # (these run while the prior RS halves are in flight);
                    # then y1 weave; then second-half pairs quarter-major so
                    # quarter 0's drain+store overlaps quarter 1's matmuls.
                    pairs = [pr for pr in order if pr is not None]
                    for pi, pr in enumerate(pairs[:half]):
                        for i in range(2):
                            z_mm(pr, i, first=(pi == 0), last=False)
                    if None in order:
                        y_emit(conv, 1,
                               xo_hf if conv == 0 else nxt_hf[(conv - 1) % 2])
                    for i in range(2):
                        for pi, pr in enumerate(pairs[half:]):
                            z_mm(pr, i, first=False,
                                 last=(pi == half - 1 if half else False))

                    # ---- drain per column-quarter i: i=0's drain+store
                    # overlaps i=1's matmuls. HW allows only ONE PSUM input
                    # per tensor-tensor op: scalar-copy the top col-tile
                    # half to SBUF, then vector-add PSUM + SBUF, then store
                    # ranks j=4i..4i+3 (on gpsimd, same queue as the RS so
                    # the trigger fences on it directly). ----
                    zt = ztpool.tile([D, CH], HF, name=f"zt{g}", tag=f"zt{g}")
                    ph = work.tile([D, 2, 512], HF, name=f"ph{g}", tag=f"ph{g}")
                    for i in range(2):
                        nc.scalar.copy(ph[:, i, :], pz[i][64:128, :])
                        nc.vector.tensor_add(
                            zt[:, i * 512:(i + 1) * 512],
                            pz[i][0:64, :], ph[:, i, :])
                        for jj in range(2):
                            j0 = 4 * i + 2 * jj
                            nc.gpsimd.dma_start(
                                cc_in[conv][g][j0:j0 + 2].transpose([1, 0, 2]),
                                zt[:, j0 * 128:(j0 + 2) * 128])

                    scc = scope(f"cc{conv}g{g}"); scc.__enter__()
                    nc.gpsimd.collective_compute(
                        "ReduceScatter",
                        mybir.AluOpType.add,
                        replica_groups=[list(range(NCORES))],
                        ins=[cc_in[conv][g].opt()],
                        outs=[cc_out[conv][g].opt()],
                    )
                    scc.__exit__(None, None, None)

                    # ---- post for owned block g: bias + (residual) + relu ----
                    gs = slice(g * 128, (g + 1) * 128)
                    emit_cb()
                    sl = work.tile([D, 128], HF, name=f"sl{g}", tag=f"sl{g}")
                    nc.sync.dma_start(sl[:, 0:64], cc_out[conv][g][:, 0:64])
                    nc.scalar.dma_start(sl[:, 64:128], cc_out[conv][g][:, 64:128])
                    t1 = work.tile([D, 128], FP, name=f"t1{g}", tag=f"t1{g}")
                    # t1 = sl * (SCALES/XSIN_next) + cb  (cb folds bias,
                    # residual x, and the x_next storage scale)
                    xs_n = XSIN[conv + 1] if conv < 3 else 1.0
                    nc.vector.scalar_tensor_tensor(
                        t1[:], sl[:], float(SCALES[conv] / xs_n),
                        cb_sb[:, conv, gs],
                        op0=mybir.AluOpType.mult, op1=mybir.AluOpType.add)
                    if conv == 3:
                        nf = work.tile([D, 128], FP, name=f"nf{g}", tag=f"nf{g}")
                        nc.vector.tensor_scalar_max(nf[:], t1[:], 0.0)
                        nc.sync.dma_start(out_d[:, gs], nf[:])
                    else:
                        nc.vector.tensor_scalar_max(
                            nxt_hf[conv % 2][:, gs], t1[:], 0.0)
                sc.__exit__(None, None, None)

    nc.compile()
    return nc


def _get_nc():
    if "nc" not in _CACHE:
        _CACHE["nc"] = _build()
    return _CACHE["nc"]


def _owned(c):
    return np.r_[128 * c:128 * c + 128, 1024 + 128 * c:1024 + 128 * c + 128]


def _prep_in_maps(inputs):
    x = np.ascontiguousarray(inputs["node_property_tensor"], dtype=np.float32)
    conn = np.ascontiguousarray(inputs["connectivity_tensor"], dtype=np.float32)
    bond = np.ascontiguousarray(inputs["bond_property_tensor"], dtype=np.float32)
    f0 = np.ascontiguousarray(inputs["filters0"], dtype=np.float32)
    f1 = np.ascontiguousarray(inputs["filters1"], dtype=np.float32)

    xT = np.ascontiguousarray(x.T)                                   # [D, A]
    fws = [f[:, :, :D].transpose(2, 1, 0).reshape(D, FO) for f in (f0, f1)]
    fw = np.concatenate(
        [fws[conv // 2] / FWDIV[conv] for conv in range(4)],
        axis=1).astype(NPHF)                                         # [D, 4*FO]
    fb = np.ascontiguousarray(np.concatenate(
        [f[:, :, D:].reshape(D, 2 * F).T for f in (f0, f1)],
        axis=1).astype(NPHF))
    bondT = np.ascontiguousarray(
        bond.transpose(1, 2, 0).reshape(2 * F, A).astype(NPHF))

    in_maps = []
    for c in range(NCORES):
        own = _owned(c)
        conn_t = np.ascontiguousarray(
            (conn[:, own, :].transpose(2, 1, 0).reshape(KL, A)
             * CONN_SCALE).astype(NPE3))
        in_maps.append({
            "conn_t": conn_t,
            "xoT_sh": np.ascontiguousarray(xT[:, own]),
            "fw": np.ascontiguousarray(fw),
            "fb": fb,
            "bondT_sh": np.ascontiguousarray(bondT[:, own]),
        })
    return in_maps


def _unshard(results):
    outT = np.empty((D, A), dtype=np.float32)
    for c in range(NCORES):
        sh = results[c]["out_sh"]
        outT[:, 128 * c:128 * c + 128] = sh[:, :128]
        outT[:, 1024 + 128 * c:1024 + 128 * c + 128] = sh[:, 128:]
    return np.ascontiguousarray(outT.T)


def kernel(node_property_tensor, connectivity_tensor, bond_property_tensor,
           filters0, filters1):
    in_maps = _prep_in_maps({
        "node_property_tensor": node_property_tensor,
        "connectivity_tensor": connectivity_tensor,
        "bond_property_tensor": bond_property_tensor,
        "filters0": filters0,
        "filters1": filters1,
    })
    nc = _get_nc()
    res = run_bass_kernel_spmd(nc, in_maps, core_ids=list(range(NCORES)))
    return _unshard(res.results)


def run_traced(in_maps, stitch=False):
    """For test.py: run with NTFF tracing, return BassKernelResults."""
    kw = {}
    if stitch:
        kw = dict(trace_cores=list(range(NCORES)), stitch_traces=True)
    return run_bass_kernel_spmd(
        _get_nc(), in_maps, core_ids=list(range(NCORES)), trace=True, **kw
    )


def make_in_maps(**inputs):
    """Expose the host-side prep for test.py tracing path."""
    return _prep_in_maps(inputs)


# revision 27
# speedup vs baseline: 1.0926x; 1.0170x over previous
"""Trainium2 Bass kernel for nn_ChemResBlock (gnn_message_passing).

Reference computation (A=2048 atoms, D=64 depth, F=12 filter slots):
    chemconv(x)[a,o] = sum_{n,f,d} conn[a,n,f] * x[n,d] * filters[o,f,d]
                       + sum_{f,c} bond[a,f,c] * filters[o,f,D+c]
    for filt in (f0, f1):
        out = relu(chemconv(out)); out = chemconv(out); out = relu(out + x)

Kernel strategy (8 NeuronCores), v7:
  * Contract-reorder: z[o,a] = sum_{n,f} conn_t[(f,n),a] * y[(f,n),o] with
    y = x @ filters — the big conn tensor is consumed by matmuls.
  * Shard the contraction (neighbor) dim across 8 cores; each core owns a
    PERMUTED set of 256 atoms: slices [128c, 128c+128) and [1024+128c, +128),
    so the two per-conv half-ReduceScatters each deliver one 128-atom slice.
  * conn stored fp8e3 * 8 (top of e3m4 normal range, immune to subnormal
    flush): halves HBM traffic + SBUF vs fp16 at the same matmul rate.
    y / z-drain / RS wire in fp16 (4x finer mantissa than bf16). Per-conv
    scales (SCALES/XSIN/FWDIV) keep fp16 wire values in range while fw
    weights stay out of denormal territory; HW rel err 2.1e-3 vs 2e-2 gate.
  * Col-tiled matmuls: chunk PAIRS run CONCURRENTLY on disjoint PE column
    halves (tile_position (0,0)/(0,64)), ~1.9x since M=64 only fills half
    the 128-wide array. Per column-quarter psum chains so quarter 0's
    drain+store overlaps quarter 1's matmuls.
  * Per conv, g-major passes: chunks for column half g (even ns0 chunks
    first, block-1 y woven mid-pass), drain -> rank-sliced stores ->
    ReduceScatter(g). RS(g0) overlaps the g1 pass; the next conv's even
    chunks run under RS(g1).
  * First-RS time = last_core_start + conv0 critical path (the CC-init
    rendezvous absorbs NEFF launch skew, then a fixed ~11us first-op
    penalty): conv0 is tuned to trigger ASAP (~37us). The combined-bias
    matmuls are deferred past conv0's z matmuls in the PE queue.
  * DMA ring discipline: sync+scalar share rings 163-172, gpsimd owns
    155-162. conn loads live on sync only; cc_in stores (4 x 32KB per
    half-conv) + RS triggers on gpsimd so they never queue behind conn.
  * Post per owned block: one scalar_tensor_tensor (wire*scale + combined
    bias incl residual) + relu(+x_next storage scale); final conv stores
    f32 shards [64, 256] (permuted atom ownership), reassembled host-side.
"""

import numpy as np
import ml_dtypes

import concourse.bacc as bacc
import concourse.bass as bass
import concourse.mybir as mybir
import concourse.tile as tile
from concourse.bass_utils import run_bass_kernel_spmd

A, D, F, NCORES = 2048, 64, 12, 8
NS = A // NCORES          # owned atoms per core = 256
KL = NS * F               # local contraction size = 3072
NCH = KL // 128           # k-chunks of 128 = 24
FO = F * D                # 768 = y columns per layer
CH = A // 2               # column half = 1024

FP = mybir.dt.float32
HF = mybir.dt.float16
E3 = mybir.dt.float8e3
NPHF = np.float16
NPE3 = ml_dtypes.float8_e3m4

# Per-conv downscale so fp16 intermediates (z partials on the RS wire) stay
# in range: activations grow ~350x per conv; conv3 pre-activations reach
# ~5.7e8 >> fp16 max. The total wire scale SCALES[conv] is split between the
# x-storage side (XSIN: x_next stored as x/XSIN) and the fw side (FWDIV kept
# <=4 so fp16 fw weights stay out of denormal/FTZ range). Un-scaled in the
# post step right after the ReduceScatter.
# conn is stored as fp8e3 * 8 (top of e3m4 normal range; immune to any HW
# subnormal flush below 0.25/8=0.03 true units), so every wire value carries
# an extra 1/8; SCALES includes that factor.
CONN_SCALE = 8.0
SCALES = [1.0, 1.0, 128.0, 65536.0]
XSIN = [1.0, 1.0, 64.0, 32768.0]
# z_wire = CONN_SCALE * z_true / (XSIN * FWDIV)  ==  z_true / SCALES
FWDIV = [s * CONN_SCALE / x for s, x in zip(SCALES, XSIN)]

_CACHE = {}


def _build():
    nc = bacc.Bacc("TRN2", target_bir_lowering=False, debug=False, num_devices=NCORES)

    conn_t_d = nc.dram_tensor("conn_t", [KL, A], E3, kind="ExternalInput").ap()
    xoT_d = nc.dram_tensor("xoT_sh", [D, NS], FP, kind="ExternalInput").ap()
    fw_d = nc.dram_tensor("fw", [D, 4 * FO], HF, kind="ExternalInput").ap()
    fb_d = nc.dram_tensor("fb", [2 * F, 2 * D], HF, kind="ExternalInput").ap()
    bondT_d = nc.dram_tensor("bondT_sh", [2 * F, NS], HF, kind="ExternalInput").ap()
    out_d = nc.dram_tensor("out_sh", [D, NS], FP, kind="ExternalOutput").ap()

    with tile.TileContext(nc) as tc:
        with (
            tc.tile_pool(name="res", bufs=1) as res_pool,
            tc.tile_pool(name="sb", bufs=1) as sb,
            tc.tile_pool(name="ypool", bufs=1) as ypool,
            tc.tile_pool(name="ztpool", bufs=2) as ztpool,
            tc.tile_pool(name="work", bufs=2) as work,
            tc.tile_pool(name="psy", bufs=2, space="PSUM") as psy,
            tc.tile_pool(name="psz", bufs=2, space="PSUM") as psz,
            tc.tile_pool(name="dram", bufs=1, space="DRAM") as dram,
        ):
            # ---- small tensors on the scalar (ACT) ring ----
            xoT_sb = sb.tile([D, NS], FP, name="xoT_sb", tag="xoT_sb")
            nc.scalar.dma_start(xoT_sb[:], xoT_d)
            fw_sb = sb.tile([D, 4 * FO], HF, name="fw_sb", tag="fw_sb")
            nc.scalar.dma_start(fw_sb[:], fw_d)
            fb_sb = sb.tile([2 * F, 2 * D], HF, name="fb_sb", tag="fb_sb")
            nc.scalar.dma_start(fb_sb[:], fb_d)
            bondT_sb = sb.tile([2 * F, NS], HF, name="bondT_sb", tag="bondT_sb")
            nc.scalar.dma_start(bondT_sb[:], bondT_d)

            # ---- conn shard: 48 half tiles [128, 1024] fp8e3, g0 halves
            # first, so conv0's g0 pass (and its ReduceScatter trigger)
            # completes after only 3.15MB of load. The first-RS start time
            # is last_core_start + conv0 critical path (the CC rendezvous
            # absorbs launch skew), so conv0 speed directly moves the
            # whole schedule. ----
            conn_res = [[None, None] for _ in range(NCH)]
            for g in range(2):
                for r in range(NCH):
                    conn_res[r][g] = res_pool.tile(
                        [128, CH], E3, name=f"cn{r}_{g}", tag=f"cn{r}_{g}")
            # all conn on the sync queue: its DMA-ring pool (163-172) is
            # shared with scalar, while gpsimd owns rings 155-162 — keeping
            # gpsimd conn-free lets the cc_in stores (and so the RS
            # triggers) complete without queueing behind conn traffic.
            for g in range(2):
                for r in range(NCH):
                    nc.sync.dma_start(
                        conn_res[r][g][:],
                        conn_t_d[r * 128:(r + 1) * 128, g * CH:(g + 1) * CH])

            # per-conv combined bias cb[conv] = (bias_layer + x?residual)/XSIN_next
            # so each post step is a single scalar_tensor_tensor + relu.
            # Emitted lazily (at conv0's first post) so the pb matmuls don't
            # sit ahead of conv0's z matmuls in the PE queue — they only
            # need to land before the first RS completes (~60us later).
            cb_sb = sb.tile([D, 4, NS], FP, name="cb_sb", tag="cb_sb")
            cb_state = {"done": False}

            def emit_cb():
                if cb_state["done"]:
                    return
                cb_state["done"] = True
                for layer in range(2):
                    pb = psy.tile([D, NS], FP, name="pb", tag="py")
                    nc.tensor.matmul(
                        pb[:], fb_sb[:, layer * D:(layer + 1) * D], bondT_sb[:],
                        start=True, stop=True,
                    )
                    ceven, codd = 2 * layer, 2 * layer + 1
                    nc.vector.tensor_scalar(
                        cb_sb[:, ceven, :], pb[:], 1.0 / XSIN[ceven + 1], None,
                        op0=mybir.AluOpType.mult)
                    xs_n = XSIN[codd + 1] if codd < 3 else 1.0
                    tmpb = work.tile([D, NS], FP, name="tmpb", tag="tmpb")
                    nc.vector.tensor_add(tmpb[:], pb[:], xoT_sb[:])
                    nc.vector.tensor_scalar(
                        cb_sb[:, codd, :], tmpb[:], 1.0 / xs_n, None,
                        op0=mybir.AluOpType.mult)

            # fp16 copy of x-transposed (conv0 y production)
            xo_hf = sb.tile([D, NS], HF, name="xo_hf", tag="xo_hf")
            nc.vector.tensor_copy(xo_hf[:], xoT_sb[:])

            y_tiles = [
                ypool.tile([128, 2, FO], HF, name=f"y{i}", tag=f"y{i}")
                for i in range(2)
            ]
            nxt_hf = [
                sb.tile([D, NS], HF, name=f"nb{i}", tag=f"nb{i}")
                for i in range(2)
            ]

            def y_emit(conv, blk, cur_hf):
                """y[n_local(blk), f, o] for this conv (pre-scaled fw copy)."""
                y_t = y_tiles[conv % 2]
                for h in range(2):
                    py = psy.tile([128, FO // 2], FP, name="py", tag="py")
                    nc.tensor.matmul(
                        py[:],
                        cur_hf[:, blk * 128:(blk + 1) * 128],
                        fw_sb[:, conv * FO + h * (FO // 2):
                              conv * FO + (h + 1) * (FO // 2)],
                        start=True, stop=True,
                    )
                    if h == 0:
                        nc.vector.tensor_copy(
                            y_t[:, blk, h * (FO // 2):(h + 1) * (FO // 2)],
                            py[:])
                    else:
                        nc.scalar.copy(
                            y_t[:, blk, h * (FO // 2):(h + 1) * (FO // 2)],
                            py[:])

            # conv0's block-0 y from x itself (block 1 woven mid-pass so
            # z matmuls start as early as possible after engine init)
            y_emit(0, 0, xo_hf)

            cc_in = [[None, None] for _ in range(4)]
            cc_out = [[None, None] for _ in range(4)]
            for c in range(4):
                for g in range(2):
                    cc_in[c][g] = dram.tile(
                        [NCORES, D, 128], HF, name=f"ci{c}{g}", tag=f"ci{c}{g}")
                    cc_out[c][g] = dram.tile(
                        [D, 128], HF, name=f"co{c}{g}", tag=f"co{c}{g}")

            scope = nc.named_scope
            for conv in range(4):
                layer = conv // 2
                y_t = y_tiles[conv % 2]
                if conv > 0:
                    # block-0 y (depends on prior conv's first RS result);
                    # lands in the PE queue after the prior conv's pass-1 MMs
                    y_emit(conv, 0, nxt_hf[(conv - 1) % 2])

                sc = scope(f"conv{conv}"); sc.__enter__()
                for g in range(2):
                    pz = [
                        psz.tile([128, 512], FP, name=f"pz{i}", tag=f"pz{i}")
                        for i in range(2)
                    ]
                    # chunk-pair order: even (ns0) chunks first so the conv
                    # can start on y-block-0 only; None marker weaves the
                    # block-1 y production mid-pass (from x for conv0, from
                    # the second RS half otherwise).
                    ev = [r for r in range(NCH) if r % 2 == 0]
                    od = [r for r in range(NCH) if r % 2 == 1]
                    order = [(ev[2 * j], ev[2 * j + 1])
                             for j in range(len(ev) // 2)]
                    if g == 0:
                        order = order + [None]
                    order = order + [(od[2 * j], od[2 * j + 1])
                                     for j in range(len(od) // 2)]
                    npair = NCH // 2
                    half = npair // 2

                    def z_mm(pr, i, first, last):
                        cs = slice(i * 512, (i + 1) * 512)
                        rA, rB = pr
                        fA, nsA = rA // 2, rA % 2
                        fB, nsB = rB // 2, rB % 2
                        for tp, rX, fX, nsX in ((0, rA, fA, nsA),
                                                (64, rB, fB, nsB)):
                            nc.tensor.matmul(
                                pz[i][tp:tp + 64, :],
                                y_t[:, nsX, fX * D:(fX + 1) * D],
                                conn_res[rX][g][:, cs],
                                start=first, stop=last,
                                tile_position=(0, tp),
                                skip_group_check=True,
                            )

                    # first-half pairs: both column quarters interleaved
                    # (these run while the prior RS halves are in flight);
                    # then y1 weave; then second-half pairs quarter-major so
                    # quarter 0's drain+store overlaps quarter 1's matmuls.
                    pairs = [pr for pr in order if pr is not None]
                    for pi, pr in enumerate(pairs[:half]):
                        for i in range(2):
                            z_mm(pr, i, first=(pi == 0), last=False)
                    if None in order:
                        y_emit(conv, 1,
                               xo_hf if conv == 0 else nxt_hf[(conv - 1) % 2])
                    for i in range(2):
                        for pi, pr in enumerate(pairs[half:]):
                            z_mm(pr, i, first=False,
                                 last=(pi == half - 1 if half else False))

                    # ---- drain per column-quarter i: i=0's drain+store
                    # overlaps i=1's matmuls. HW allows only ONE PSUM input
                    # per tensor-tensor op: scalar-copy the top col-tile
                    # half to SBUF, then vector-add PSUM + SBUF, then store
                    # ranks j=4i..4i+3 (on gpsimd, same queue as the RS so
                    # the trigger fences on it directly). ----
                    zt = ztpool.tile([D, CH], HF, name=f"zt{g}", tag=f"zt{g}")
                    ph = work.tile([D, 2, 512], HF, name=f"ph{g}", tag=f"ph{g}")
                    for i in range(2):
                        nc.scalar.copy(ph[:, i, :], pz[i][64:128, :])
                        nc.vector.tensor_add(
                            zt[:, i * 512:(i + 1) * 512],
                            pz[i][0:64, :], ph[:, i, :])
                        for jj in range(2):
                            j0 = 4 * i + 2 * jj
                            nc.gpsimd.dma_start(
                                cc_in[conv][g][j0:j0 + 2].transpose([1, 0, 2]),
                                zt[:, j0 * 128:(j0 + 2) * 128])

                    scc = scope(f"cc{conv}g{g}"); scc.__enter__()
                    nc.gpsimd.collective_compute(
                        "ReduceScatter",
                        mybir.AluOpType.add,
                        replica_groups=[list(range(NCORES))],
                        ins=[cc_in[conv][g].opt()],
                        outs=[cc_out[conv][g].opt()],
                    )
                    scc.__exit__(None, None, None)

                    # ---- post for owned block g: bias + (residual) + relu ----
                    gs = slice(g * 128, (g + 1) * 128)
                    emit_cb()
                    sl = work.tile([D, 128], HF, name=f"sl{g}", tag=f"sl{g}")
                    nc.sync.dma_start(sl[:, 0:64], cc_out[conv][g][:, 0:64])
                    nc.scalar.dma_start(sl[:, 64:128], cc_out[conv][g][:, 64:128])
                    t1 = work.tile([D, 128], FP, name=f"t1{g}", tag=f"t1{g}")
                    # t1 = sl * (SCALES/XSIN_next) + cb  (cb folds bias,
                    # residual x, and the x_next storage scale)
                    xs_n = XSIN[conv + 1] if conv < 3 else 1.0
                    nc.vector.scalar_tensor_tensor(
                        t1[:], sl[:], float(SCALES[conv] / xs_n),
                        cb_sb[:, conv, gs],
                        op0=mybir.AluOpType.mult, op1=mybir.AluOpType.add)
                    if conv == 3:
                        nf = work.tile([D, 128], FP, name=f"nf{g}", tag=f"nf{g}")
                        nc.vector.tensor_scalar_max(nf[:], t1[:], 0.0)
                        nc.sync.dma_start(out_d[:, gs], nf[:])
                    else:
                        nc.vector.tensor_scalar_max(
                            nxt_hf[conv % 2][:, gs], t1[:], 0.0)
                sc.__exit__(None, None, None)

    nc.compile()
    return nc


def _get_nc():
    if "nc" not in _CACHE:
        _CACHE["nc"] = _build()
    return _CACHE["nc"]


def _owned(c):
    return np.r_[128 * c:128 * c + 128, 1024 + 128 * c:1024 + 128 * c + 128]


def _prep_in_maps(inputs):
    x = np.ascontiguousarray(inputs["node_property_tensor"], dtype=np.float32)
    conn = np.ascontiguousarray(inputs["connectivity_tensor"], dtype=np.float32)
    bond = np.ascontiguousarray(inputs["bond_property_tensor"], dtype=np.float32)
    f0 = np.ascontiguousarray(inputs["filters0"], dtype=np.float32)
    f1 = np.ascontiguousarray(inputs["filters1"], dtype=np.float32)

    xT = np.ascontiguousarray(x.T)                                   # [D, A]
    fws = [f[:, :, :D].transpose(2, 1, 0).reshape(D, FO) for f in (f0, f1)]
    fw = np.concatenate(
        [fws[conv // 2] / FWDIV[conv] for conv in range(4)],
        axis=1).astype(NPHF)                                         # [D, 4*FO]
    fb = np.ascontiguousarray(np.concatenate(
        [f[:, :, D:].reshape(D, 2 * F).T for f in (f0, f1)],
        axis=1).astype(NPHF))
    bondT = np.ascontiguousarray(
        bond.transpose(1, 2, 0).reshape(2 * F, A).astype(NPHF))

    in_maps = []
    for c in range(NCORES):
        own = _owned(c)
        conn_t = np.ascontiguousarray(
            (conn[:, own, :].transpose(2, 1, 0).reshape(KL, A)
             * CONN_SCALE).astype(NPE3))
        in_maps.append({
            "conn_t": conn_t,
            "xoT_sh": np.ascontiguousarray(xT[:, own]),
            "fw": np.ascontiguousarray(fw),
            "fb": fb,
            "bondT_sh": np.ascontiguousarray(bondT[:, own]),
        })
    return in_maps


def _unshard(results):
    outT = np.empty((D, A), dtype=np.float32)
    for c in range(NCORES):
        sh = results[c]["out_sh"]
        outT[:, 128 * c:128 * c + 128] = sh[:, :128]
        outT[:, 1024 + 128 * c:1024 + 128 * c + 128] = sh[:, 128:]
    return np.ascontiguousarray(outT.T)


def kernel(node_property_tensor, connectivity_tensor, bond_property_tensor,
           filters0, filters1):
    in_maps = _prep_in_maps({
        "node_property_tensor": node_property_tensor,
        "connectivity_tensor": connectivity_tensor,
        "bond_property_tensor": bond_property_tensor,
        "filters0": filters0,
        "filters1": filters1,
    })
    nc = _get_nc()
    res = run_bass_kernel_spmd(nc, in_maps, core_ids=list(range(NCORES)))
    return _unshard(res.results)


def run_traced(in_maps, stitch=False):
    """For test.py: run with NTFF tracing, return BassKernelResults."""
    kw = {}
    if stitch:
        kw = dict(trace_cores=list(range(NCORES)), stitch_traces=True)
    return run_bass_kernel_spmd(
        _get_nc(), in_maps, core_ids=list(range(NCORES)), trace=True, **kw
    )


def make_in_maps(**inputs):
    """Expose the host-side prep for test.py tracing path."""
    return _prep_in_maps(inputs)
